# revision 25
# baseline (speedup 1.0000x reference)
"""Channel-attention (per-head [64,64] score matrix) Trainium2 Bass kernel.

Math (per batch b of 16):
    qkv = x @ w_qkv                 # x [4096, 256], w_qkv [256, 1536]
    q,k,v = split(qkv); per head h (8 heads x 64 dim):
    sim_h = (q_h * 8^-1)^T @ k_h    # [64, 64]   (contracts spatial d=4096)
    attn_h = softmax(sim_h, axis=-1)
    out_h = v_h @ attn_h^T          # [4096, 64]
    y = concat(out_h) @ w_out + b_out

Distribution: data-parallel over batch — 8 cores x 2 batches each; weights
replicated; no collectives. The host pre-transposes x to [C, d] per batch so
every device matmul streams with large free dims, pre-folds the 1/8 q-scale
into w_q, pre-converts inputs to fp16 (all matmuls run fp16 x fp16 with fp32
PSUM accumulation; end-to-end rel-l2 ~1.6e-3 vs fp64 oracle), and adds the
output bias on the host (so y can DMA straight out of PSUM).

Device dataflow per batch (phases ordered so V-phase matmuls hide the
softmax latency on PE):
  QK:   q,k [d-chunk 128, 512each] (lhsT = xT d-chunk, rhs = w_qk cols, N=512)
  B:    sim[p] [128,128] per head-pair accumulates over 32 d-chunks
  V:    vT[m,d] = w_v.T @ xT       (lhsT = w_v chunk, rhs = xT d-cols, N=512)
  soft: rowmax (negated) -> exp(sim - max) with accum_out row-sums ->
        recip -> scale e rows by 1/s (so C1's PSUM drain is a plain copy)
  T:    PE-transpose e_p -> eT_p (C1's stationary operand)
  C1:   outT[i,d] = eT_h @ vT_h, two heads per PE pass (row/col split)
  C2:   y[d,c] = outT.T @ w_out, DMA'd to HBM directly from PSUM (fp32)
"""

import numpy as np

import concourse.bass as bass
import concourse.mybir as mybir
from concourse.bass_utils import run_bass_kernel_spmd
from concourse.masks import make_identity
from concourse.tile import TileContext


def _split_multi_waits(nc, limit=1):
    """Post-pass: the walrus build in this container rejects instructions
    carrying more than `limit` sync-waits ("Too many sync wait commands" in
    setupSyncWait). Tile attaches up to 3. Hoist the extras onto same-engine
    NoOp instructions inserted immediately before the owner — the engine
    sequencer executes them in order, so the ordering semantics are
    identical (single-wait instructions are what the rest of the Tile
    output uses, and those compile)."""
    n_split = 0
    for f in nc.m.functions:
        for blk in f.blocks:
            il = blk.instructions
            i = 0
            while i < len(il):
                inst = il[i]
                si = inst.sync_info
                waits = list(si.on_wait) if si is not None else []
                if len(waits) > limit:
                    si.on_wait = waits[:limit]
                    for w in waits[limit:]:
                        nop = mybir.InstNoOp(
                            name=f"I-waitsplit-{n_split}", ins=[], outs=[]
                        )
                        n_split += 1
                        nop.engine = inst.engine
                        nop.sync_info = mybir.SyncInfo(on_wait=[w], on_update=[])
                        il.insert(i, nop)
                        i += 1
                i += 1
    return nc


N_CORES = 8
BATCH = 16
BPC = BATCH // N_CORES  # batches per core
D = 4096  # spatial (64*64)
C = 256   # channels
HID = 512
HEADS = 8
DH = 64

F32 = mybir.dt.float32
F16 = mybir.dt.float16

_CACHE = {}


def _build():
    nc = bass.Bass()
    xT_d = nc.declare_dram_parameter("xT", [BPC, C, D], F16, isOutput=False)
    wqkv_d = nc.declare_dram_parameter("w_qkv", [C, 3 * HID], F16, isOutput=False)
    wout_d = nc.declare_dram_parameter("w_out_r", [128, 4, C], F16, isOutput=False)
    y_d = nc.declare_dram_parameter("y", [BPC, D, C], F32, isOutput=True)

    with TileContext(nc) as tc:
        with (
            tc.tile_pool(name="consts", bufs=1) as consts,
            tc.tile_pool(name="xt", bufs=2) as xt_pool,
            tc.tile_pool(name="vt", bufs=8) as vt_pool,
            tc.tile_pool(name="qk", bufs=6) as qk_pool,
            tc.tile_pool(name="eP", bufs=8) as e_pool,
            tc.tile_pool(name="stat", bufs=6) as stat_pool,
            tc.tile_pool(name="ot", bufs=8) as ot_pool,
            tc.tile_pool(name="ysb", bufs=4) as y_pool,
            tc.tile_pool(name="mm", bufs=6, space="PSUM") as mm_pool,
            tc.tile_pool(name="simp", bufs=2, space="PSUM") as sim_pool,
        ):
            # ---- constants ----
            # w_qkv split loads ordered by first use: w_q, then w_k, then
            # w_v / w_out (V and C2 run much later).
            w_sb = []
            for ci in range(2):
                w_t = consts.tile([128, 3 * HID], F16, name=f"w{ci}")
                w_sb.append(w_t)
            for lo, hi in ((0, HID), (HID, 2 * HID)):
                for ci in range(2):
                    nc.sync.dma_start(
                        out=w_sb[ci][:, lo:hi],
                        in_=wqkv_d[ci * 128:(ci + 1) * 128, lo:hi],
                    )
            wo_sb = consts.tile([128, 4, C], F16, name="wo")
            ident = consts.tile([128, 128], F32, name="ident")
            make_identity(nc, ident)

            for b in range(BPC):
                # ---- load xT (chunked so the first QK matmuls start early) --
                xt = []
                for ci in range(2):
                    x_t = xt_pool.tile([128, D], F16, name=f"xt{ci}", tag="xt")
                    xt.append(x_t)
                # first 512 cols arrive alone so QK d1=0..3 can start early
                chunks = [(0, 512)] + [(lo, lo + 896) for lo in range(512, D, 896)]
                for lo, hi in chunks:
                    hi = min(hi, D)
                    for ci in range(2):
                        nc.sync.dma_start(
                            out=xt[ci][:, lo:hi],
                            in_=xT_d[b, ci * 128:(ci + 1) * 128, lo:hi],
                        )

                # ---- phase QK + B ----
                # sim[p]: one PSUM bank per accumulation group (start=True
                # zeroes a whole 2KB zero-region per written partition, so
                # groups must not share a bank). Tile p = head pair
                # (2p, 2p+1): rows i (head 2p at 0:64, 2p+1 at 64:128),
                # cols j likewise; diag 64x64 blocks are the per-head sims.
                # sim_all [128, 256]: ONE psum bank holds all 8 per-head
                # accumulators — pair p at cols p*64:+64, head 2p at rows
                # 0:64, head 2p+1 at rows 64:128. The bank is zeroed by an
                # explicit memset and every matmul uses start=False
                # (accumulate) — order-independent, so the scheduler may
                # interleave the groups freely.
                sim_all = sim_pool.tile([128, 256], F32, name="sim_all", tag="simp")
                nc.vector.memset(sim_all, 0.0)
                def emit_b(qk_tile, d1):
                    # sim matmuls for the qk tile of iteration d1 (emitted one
                    # iteration late so the PSUM->SBUF copy latency hides
                    # under the next iteration's qk matmuls)
                    for p in range(4):
                        for par in range(2):
                            q_lo = p * 128 + par * 64
                            nc.tensor.matmul(
                                sim_all[par * 64:(par + 1) * 64, p * 64:(p + 1) * 64],
                                lhsT=qk_tile[:, q_lo:q_lo + 64],
                                rhs=qk_tile[:, 512 + q_lo:512 + q_lo + 64],
                                start=False,
                                stop=(d1 == 31),
                                skip_group_check=True,
                            )

                prev = None
                for d1 in range(32):
                    qps = mm_pool.tile([128, 512], F32, name="qps", tag="mm")
                    kps = mm_pool.tile([128, 512], F32, name="kps", tag="mm")
                    for ci in range(2):
                        nc.tensor.matmul(
                            qps,
                            lhsT=xt[ci][:, d1 * 128:(d1 + 1) * 128],
                            rhs=w_sb[ci][:, 0:HID],
                            start=(ci == 0),
                            stop=(ci == 1),
                        )
                    for ci in range(2):
                        nc.tensor.matmul(
                            kps,
                            lhsT=xt[ci][:, d1 * 128:(d1 + 1) * 128],
                            rhs=w_sb[ci][:, HID:2 * HID],
                            start=(ci == 0),
                            stop=(ci == 1),
                        )
                    qk = qk_pool.tile([128, 1024], F16, name="qk", tag="qk")
                    nc.any.tensor_copy(qk[:, 0:512], qps)
                    nc.any.tensor_copy(qk[:, 512:1024], kps)
                    if prev is not None:
                        emit_b(*prev)
                    prev = (qk, d1)

                # ---- phase V (PE work that hides softmax latency) ----
                # d5-outer so vt[0..3] become ready column-range by
                # column-range — C1's d5 loop can start at d5=0 early. The
                # first d5 iteration is emitted BEFORE the last deferred B
                # matmuls so the scheduler has PE work to cover the final
                # qk copy's latency.
                if b == 0:
                    # deferred weight loads (not needed until now)
                    for ci in range(2):
                        nc.sync.dma_start(
                            out=w_sb[ci][:, 2 * HID:3 * HID],
                            in_=wqkv_d[ci * 128:(ci + 1) * 128, 2 * HID:3 * HID],
                        )
                    nc.sync.dma_start(out=wo_sb, in_=wout_d[:, :, :])
                vt = []
                for m in range(4):
                    v_t = vt_pool.tile([128, D], F16, name=f"vt{m}", tag="vt")
                    vt.append(v_t)

                def emit_v(d5):
                    for m in range(4):
                        wv_lo = 2 * HID + m * 128
                        vps = mm_pool.tile([128, 512], F32, name="vps", tag="mm")
                        for ci in range(2):
                            nc.tensor.matmul(
                                vps,
                                lhsT=w_sb[ci][:, wv_lo:wv_lo + 128],
                                rhs=xt[ci][:, d5 * 512:(d5 + 1) * 512],
                                start=(ci == 0),
                                stop=(ci == 1),
                            )
                        nc.any.tensor_copy(vt[m][:, d5 * 512:(d5 + 1) * 512], vps)

                emit_b(*prev)
                for d5 in range(8):
                    emit_v(d5)

                # ---- softmax (DVE/ACT; overlaps V on PE) ----
                # head h: pair p=h//2, par=h%2; diag block of sim[p] at
                # rows/cols par*64:+64.
                m_t = stat_pool.tile([128, 4], F32, name="m_t", tag="stat")
                s_t = stat_pool.tile([128, 4], F32, name="s_t", tag="stat")
                r_t = stat_pool.tile([128, 4], F32, name="r_t", tag="stat")
                e_tiles = []
                for p in range(4):
                    e_p = e_pool.tile([128, 128], F32, name=f"e{p}", tag="e")
                    nc.gpsimd.memset(e_p, 0.0)
                    e_tiles.append(e_p)
                for h in range(HEADS):
                    par, p = h % 2, h // 2
                    rows = slice(par * 64, par * 64 + 64)
                    nc.vector.reduce_max(
                        out=m_t[rows, p:p + 1],
                        in_=sim_all[rows, p * 64:(p + 1) * 64],
                        axis=mybir.AxisListType.X,
                        negate=True,
                    )
                for h in range(HEADS):
                    par, p = h % 2, h // 2
                    rows = slice(par * 64, par * 64 + 64)
                    nc.scalar.activation(
                        out=e_tiles[p][rows, par * 64:par * 64 + 64],
                        in_=sim_all[rows, p * 64:(p + 1) * 64],
                        func=mybir.ActivationFunctionType.Exp,
                        bias=m_t[rows, p:p + 1],
                        scale=1.0,
                        accum_out=s_t[rows, p:p + 1],
                    )
                nc.vector.reciprocal(r_t, s_t)
                # attn = e / s: fold 1/s into e rows now (tiny [128,128]
                # tiles) instead of scaling every [128,512] C1 output.
                for p in range(4):
                    nc.vector.tensor_scalar_mul(
                        e_tiles[p], e_tiles[p], r_t[:, p:p + 1]
                    )

                # ---- transpose e -> eT (PE) ----
                eT_tiles = []
                for p in range(4):
                    etps = mm_pool.tile([128, 128], F32, name="etps", tag="mm")
                    nc.tensor.transpose(etps, e_tiles[p], ident)
                    eT_s = e_pool.tile([128, 128], F16, name=f"eT{p}", tag="eT")
                    nc.any.tensor_copy(eT_s, etps)
                    eT_tiles.append(eT_s)

                # ---- phase C: attention-apply + output projection ----
                def emit_c2(ot_tiles, d5):
                    # C2 for d5's ot tiles (emitted one d5 late so the ot
                    # copy latency hides under the next d5's C1 matmuls)
                    for d1 in range(4):
                        yps = mm_pool.tile([128, C], F32, name="yps", tag="mm")
                        for p4 in range(4):
                            nc.tensor.matmul(
                                yps,
                                lhsT=ot_tiles[p4][:, d1 * 128:(d1 + 1) * 128],
                                rhs=wo_sb[:, p4, :],
                                start=(p4 == 0),
                                stop=(p4 == 3),
                            )
                        ysb = y_pool.tile([128, C], F32, name="ysb", tag="ysb")
                        nc.any.tensor_copy(ysb, yps)
                        d_lo = d5 * 512 + d1 * 128
                        nc.sync.dma_start(out=y_d[b, d_lo:d_lo + 128, :], in_=ysb)

                prev_c = None
                for d5 in range(8):
                    ot_tiles = []
                    for p in range(4):
                        c1ps = mm_pool.tile([128, 512], F32, name="c1ps", tag="mm")
                        for par in range(2):
                            rs = slice(par * 64, par * 64 + 64)
                            nc.tensor.matmul(
                                c1ps[rs, :],
                                lhsT=eT_tiles[p][rs, par * 64:par * 64 + 64],
                                rhs=vt[p][rs, d5 * 512:(d5 + 1) * 512],
                                start=True,
                                stop=True,
                            )
                        ot = ot_pool.tile([128, 512], F16, name=f"ot{p}", tag="ot")
                        nc.any.tensor_copy(ot, c1ps)
                        ot_tiles.append(ot)
                    if prev_c is not None:
                        emit_c2(*prev_c)
                    prev_c = (ot_tiles, d5)
                emit_c2(*prev_c)
    return _split_multi_waits(nc)


def _get_nc():
    if "nc" not in _CACHE:
        _CACHE["nc"] = _build()
    return _CACHE["nc"]


def kernel(x, w_qkv, w_out, b_out, **kw):
    x = np.asarray(x, dtype=np.float32)
    w_qkv = np.asarray(w_qkv, dtype=np.float32)
    w_out = np.asarray(w_out, dtype=np.float32)
    b_out = np.asarray(b_out, dtype=np.float32)

    # fold q-scale into w_q (exact: power-of-two scale), then fp16-quantize
    w_qkv_s = w_qkv.copy()
    w_qkv_s[:, :HID] *= DH ** (-0.5)
    w_qkv_s = np.ascontiguousarray(w_qkv_s.astype(np.float16))
    # w_out [512, 256] -> [128, 4, 256] with [p, t, c] = w_out[t*128+p, c]
    w_out_r = np.ascontiguousarray(
        w_out.reshape(4, 128, C).transpose(1, 0, 2).astype(np.float16)
    )

    x4 = x.reshape(BATCH, D, C).astype(np.float16)
    in_maps = []
    for core in range(N_CORES):
        xs = np.ascontiguousarray(
            x4[core * BPC:(core + 1) * BPC].transpose(0, 2, 1)
        )  # [BPC, C, D] fp16
        in_maps.append({"xT": xs, "w_qkv": w_qkv_s, "w_out_r": w_out_r})

    nc = _get_nc()
    res = run_bass_kernel_spmd(nc, in_maps, core_ids=list(range(N_CORES)), **kw)
    y = np.concatenate([r["y"] for r in res.results], axis=0)  # [16, 4096, 256]
    y += b_out  # bias on host (broadcast over last axis)
    return y.reshape(BATCH, 64, 64, C)


# revision 26
# speedup vs baseline: 1.0340x; 1.0340x over previous
"""Channel-attention (per-head [64,64] score matrix) Trainium2 Bass kernel.

Math (per batch b of 16):
    qkv = x @ w_qkv                 # x [4096, 256], w_qkv [256, 1536]
    q,k,v = split(qkv); per head h (8 heads x 64 dim):
    sim_h = (q_h * 8^-1)^T @ k_h    # [64, 64]   (contracts spatial d=4096)
    attn_h = softmax(sim_h, axis=-1)
    out_h = v_h @ attn_h^T          # [4096, 64]
    y = concat(out_h) @ w_out + b_out

Distribution: data-parallel over batch — 8 cores x 2 batches each; weights
replicated; no collectives. The host pre-transposes x to [C, d] per batch so
every device matmul streams with large free dims, pre-folds the 1/8 q-scale
into w_q, pre-converts inputs to fp16 (all matmuls run fp16 x fp16 with fp32
PSUM accumulation; end-to-end rel-l2 ~1.6e-3 vs fp64 oracle), and adds the
output bias on the host (so y can DMA straight out of PSUM).

Device dataflow per batch (phases ordered so V-phase matmuls hide the
softmax latency on PE):
  QK:   q,k [d-chunk 128, 512each] (lhsT = xT d-chunk, rhs = w_qk cols, N=512)
  B:    sim[p] [128,128] per head-pair accumulates over 32 d-chunks
  V:    vT[m,d] = w_v.T @ xT       (lhsT = w_v chunk, rhs = xT d-cols, N=512)
  soft: rowmax (negated) -> exp(sim - max) with accum_out row-sums ->
        recip -> scale e rows by 1/s (so C1's PSUM drain is a plain copy)
  T:    PE-transpose e_p -> eT_p (C1's stationary operand)
  C1:   outT[i,d] = eT_h @ vT_h, two heads per PE pass (row/col split)
  C2:   y[d,c] = outT.T @ w_out, DMA'd to HBM directly from PSUM (fp32)
"""

import numpy as np

import concourse.bass as bass
import concourse.mybir as mybir
from concourse.bass_utils import run_bass_kernel_spmd
from concourse.masks import make_identity
from concourse.tile import TileContext


def _split_multi_waits(nc, limit=1):
    """Post-pass: the walrus build in this container rejects instructions
    carrying more than `limit` sync-waits ("Too many sync wait commands" in
    setupSyncWait). Tile attaches up to 3. Hoist the extras onto same-engine
    NoOp instructions inserted immediately before the owner — the engine
    sequencer executes them in order, so the ordering semantics are
    identical (single-wait instructions are what the rest of the Tile
    output uses, and those compile)."""
    n_split = 0
    for f in nc.m.functions:
        for blk in f.blocks:
            il = blk.instructions
            i = 0
            while i < len(il):
                inst = il[i]
                si = inst.sync_info
                waits = list(si.on_wait) if si is not None else []
                if len(waits) > limit:
                    si.on_wait = waits[:limit]
                    for w in waits[limit:]:
                        nop = mybir.InstNoOp(
                            name=f"I-waitsplit-{n_split}", ins=[], outs=[]
                        )
                        n_split += 1
                        nop.engine = inst.engine
                        nop.sync_info = mybir.SyncInfo(on_wait=[w], on_update=[])
                        il.insert(i, nop)
                        i += 1
                i += 1
    return nc


N_CORES = 8
BATCH = 16
BPC = BATCH // N_CORES  # batches per core
D = 4096  # spatial (64*64)
C = 256   # channels
HID = 512
HEADS = 8
DH = 64

F32 = mybir.dt.float32
F16 = mybir.dt.float16

_CACHE = {}


def _build():
    nc = bass.Bass()
    xT_d = nc.declare_dram_parameter("xT", [BPC, C, D], F16, isOutput=False)
    wqkv_d = nc.declare_dram_parameter("w_qkv", [C, 3 * HID], F16, isOutput=False)
    wout_d = nc.declare_dram_parameter("w_out_r", [128, 4, C], F16, isOutput=False)
    y_d = nc.declare_dram_parameter("y", [BPC, D, C], F32, isOutput=True)

    with TileContext(nc) as tc:
        with (
            tc.tile_pool(name="consts", bufs=1) as consts,
            tc.tile_pool(name="xt", bufs=2) as xt_pool,
            tc.tile_pool(name="vt", bufs=8) as vt_pool,
            tc.tile_pool(name="qk", bufs=6) as qk_pool,
            tc.tile_pool(name="eP", bufs=8) as e_pool,
            tc.tile_pool(name="stat", bufs=6) as stat_pool,
            tc.tile_pool(name="ot", bufs=8) as ot_pool,
            tc.tile_pool(name="ysb", bufs=4) as y_pool,
            tc.tile_pool(name="mm", bufs=6, space="PSUM") as mm_pool,
            tc.tile_pool(name="simp", bufs=2, space="PSUM") as sim_pool,
        ):
            # ---- constants ----
            # w_qkv split loads ordered by first use: w_q, then w_k, then
            # w_v / w_out (V and C2 run much later).
            w_sb = []
            for ci in range(2):
                w_t = consts.tile([128, 3 * HID], F16, name=f"w{ci}")
                w_sb.append(w_t)
            for lo, hi in ((0, HID), (HID, 2 * HID)):
                for ci in range(2):
                    nc.sync.dma_start(
                        out=w_sb[ci][:, lo:hi],
                        in_=wqkv_d[ci * 128:(ci + 1) * 128, lo:hi],
                    )
            wo_sb = consts.tile([128, 4, C], F16, name="wo")
            ident = consts.tile([128, 128], F32, name="ident")
            make_identity(nc, ident)

            for b in range(BPC):
                # ---- load xT (chunked so the first QK matmuls start early) --
                xt = []
                for ci in range(2):
                    x_t = xt_pool.tile([128, D], F16, name=f"xt{ci}", tag="xt")
                    xt.append(x_t)
                # first 512 cols arrive alone so QK d1=0..3 can start early
                chunks = [(0, 512)] + [(lo, lo + 896) for lo in range(512, D, 896)]
                for lo, hi in chunks:
                    hi = min(hi, D)
                    for ci in range(2):
                        nc.sync.dma_start(
                            out=xt[ci][:, lo:hi],
                            in_=xT_d[b, ci * 128:(ci + 1) * 128, lo:hi],
                        )

                # ---- phase QK + B ----
                # sim[p]: one PSUM bank per accumulation group (start=True
                # zeroes a whole 2KB zero-region per written partition, so
                # groups must not share a bank). Tile p = head pair
                # (2p, 2p+1): rows i (head 2p at 0:64, 2p+1 at 64:128),
                # cols j likewise; diag 64x64 blocks are the per-head sims.
                # sim_all [128, 256]: ONE psum bank holds all 8 per-head
                # accumulators — pair p at cols p*64:+64, head 2p at rows
                # 0:64, head 2p+1 at rows 64:128. The bank is zeroed by an
                # explicit memset and every matmul uses start=False
                # (accumulate) — order-independent, so the scheduler may
                # interleave the groups freely.
                sim_all = sim_pool.tile([128, 256], F32, name="sim_all", tag="simp")
                nc.vector.memset(sim_all, 0.0)
                def emit_b(qk_tile, d1):
                    # sim matmuls for the qk tile of iteration d1 (emitted one
                    # iteration late so the PSUM->SBUF copy latency hides
                    # under the next iteration's qk matmuls)
                    for p in range(4):
                        for par in range(2):
                            q_lo = p * 128 + par * 64
                            nc.tensor.matmul(
                                sim_all[par * 64:(par + 1) * 64, p * 64:(p + 1) * 64],
                                lhsT=qk_tile[:, q_lo:q_lo + 64],
                                rhs=qk_tile[:, 512 + q_lo:512 + q_lo + 64],
                                start=False,
                                stop=(d1 == 31),
                                skip_group_check=True,
                            )

                prev = None
                for d1 in range(32):
                    qps = mm_pool.tile([128, 512], F32, name="qps", tag="mm")
                    kps = mm_pool.tile([128, 512], F32, name="kps", tag="mm")
                    for ci in range(2):
                        nc.tensor.matmul(
                            qps,
                            lhsT=xt[ci][:, d1 * 128:(d1 + 1) * 128],
                            rhs=w_sb[ci][:, 0:HID],
                            start=(ci == 0),
                            stop=(ci == 1),
                        )
                    for ci in range(2):
                        nc.tensor.matmul(
                            kps,
                            lhsT=xt[ci][:, d1 * 128:(d1 + 1) * 128],
                            rhs=w_sb[ci][:, HID:2 * HID],
                            start=(ci == 0),
                            stop=(ci == 1),
                        )
                    qk = qk_pool.tile([128, 1024], F16, name="qk", tag="qk")
                    nc.any.tensor_copy(qk[:, 0:512], qps)
                    nc.any.tensor_copy(qk[:, 512:1024], kps)
                    if prev is not None:
                        emit_b(*prev)
                    prev = (qk, d1)

                # ---- phase V (PE work that hides softmax latency) ----
                # d5-outer so vt[0..3] become ready column-range by
                # column-range — C1's d5 loop can start at d5=0 early. The
                # first d5 iteration is emitted BEFORE the last deferred B
                # matmuls so the scheduler has PE work to cover the final
                # qk copy's latency.
                if b == 0:
                    # deferred weight loads (not needed until now)
                    for ci in range(2):
                        nc.sync.dma_start(
                            out=w_sb[ci][:, 2 * HID:3 * HID],
                            in_=wqkv_d[ci * 128:(ci + 1) * 128, 2 * HID:3 * HID],
                        )
                    nc.sync.dma_start(out=wo_sb, in_=wout_d[:, :, :])
                vt = []
                for m in range(4):
                    v_t = vt_pool.tile([128, D], F16, name=f"vt{m}", tag="vt")
                    vt.append(v_t)

                def emit_v(d5):
                    for m in range(4):
                        wv_lo = 2 * HID + m * 128
                        vps = mm_pool.tile([128, 512], F32, name="vps", tag="mm")
                        for ci in range(2):
                            nc.tensor.matmul(
                                vps,
                                lhsT=w_sb[ci][:, wv_lo:wv_lo + 128],
                                rhs=xt[ci][:, d5 * 512:(d5 + 1) * 512],
                                start=(ci == 0),
                                stop=(ci == 1),
                            )
                        nc.any.tensor_copy(vt[m][:, d5 * 512:(d5 + 1) * 512], vps)

                emit_b(*prev)
                for d5 in range(8):
                    emit_v(d5)

                # ---- softmax (DVE/ACT; overlaps V on PE) ----
                # head h: pair p=h//2, par=h%2; diag block of sim[p] at
                # rows/cols par*64:+64.
                m_t = stat_pool.tile([128, 4], F32, name="m_t", tag="stat")
                s_t = stat_pool.tile([128, 4], F32, name="s_t", tag="stat")
                r_t = stat_pool.tile([128, 4], F32, name="r_t", tag="stat")
                e_tiles = []
                for p in range(4):
                    e_p = e_pool.tile([128, 128], F32, name=f"e{p}", tag="e")
                    nc.gpsimd.memset(e_p, 0.0)
                    e_tiles.append(e_p)
                for h in range(HEADS):
                    par, p = h % 2, h // 2
                    rows = slice(par * 64, par * 64 + 64)
                    nc.vector.reduce_max(
                        out=m_t[rows, p:p + 1],
                        in_=sim_all[rows, p * 64:(p + 1) * 64],
                        axis=mybir.AxisListType.X,
                        negate=True,
                    )
                for h in range(HEADS):
                    par, p = h % 2, h // 2
                    rows = slice(par * 64, par * 64 + 64)
                    nc.scalar.activation(
                        out=e_tiles[p][rows, par * 64:par * 64 + 64],
                        in_=sim_all[rows, p * 64:(p + 1) * 64],
                        func=mybir.ActivationFunctionType.Exp,
                        bias=m_t[rows, p:p + 1],
                        scale=1.0,
                        accum_out=s_t[rows, p:p + 1],
                    )
                nc.vector.reciprocal(r_t, s_t)
                # attn = e / s: fold 1/s into e rows now (tiny [128,128]
                # tiles) instead of scaling every [128,512] C1 output.
                for p in range(4):
                    nc.vector.tensor_scalar_mul(
                        e_tiles[p], e_tiles[p], r_t[:, p:p + 1]
                    )

                # ---- transpose e -> eT (PE) ----
                eT_tiles = []
                for p in range(4):
                    etps = mm_pool.tile([128, 128], F32, name="etps", tag="mm")
                    nc.tensor.transpose(etps, e_tiles[p], ident)
                    eT_s = e_pool.tile([128, 128], F16, name=f"eT{p}", tag="eT")
                    nc.any.tensor_copy(eT_s, etps)
                    eT_tiles.append(eT_s)

                # ---- phase C: attention-apply + output projection ----
                def emit_c2(ot_tiles, d5):
                    # C2 for d5's ot tiles (emitted one d5 late so the ot
                    # copy latency hides under the next d5's C1 matmuls)
                    for d1 in range(4):
                        yps = mm_pool.tile([128, C], F32, name="yps", tag="mm")
                        for p4 in range(4):
                            nc.tensor.matmul(
                                yps,
                                lhsT=ot_tiles[p4][:, d1 * 128:(d1 + 1) * 128],
                                rhs=wo_sb[:, p4, :],
                                start=(p4 == 0),
                                stop=(p4 == 3),
                            )
                        ysb = y_pool.tile([128, C], F32, name="ysb", tag="ysb")
                        nc.any.tensor_copy(ysb, yps)
                        d_lo = d5 * 512 + d1 * 128
                        nc.sync.dma_start(out=y_d[b, d_lo:d_lo + 128, :], in_=ysb)

                prev_c = None
                for d5 in range(8):
                    ot_tiles = []
                    for p in range(4):
                        c1ps = mm_pool.tile([128, 512], F32, name="c1ps", tag="mm")
                        # eT_p is exactly block-diagonal (off-diag blocks are
                        # memset zeros), so one full-array K=128 matmul
                        # computes both heads: rows 0:64 of eT only meet
                        # vt rows 0:64 (head 2p), rows 64:128 only head 2p+1.
                        nc.tensor.matmul(
                            c1ps,
                            lhsT=eT_tiles[p],
                            rhs=vt[p][:, d5 * 512:(d5 + 1) * 512],
                            start=True,
                            stop=True,
                        )
                        ot = ot_pool.tile([128, 512], F16, name=f"ot{p}", tag="ot")
                        nc.any.tensor_copy(ot, c1ps)
                        ot_tiles.append(ot)
                    if prev_c is not None:
                        emit_c2(*prev_c)
                    prev_c = (ot_tiles, d5)
                emit_c2(*prev_c)
    return _split_multi_waits(nc)


def _get_nc():
    if "nc" not in _CACHE:
        _CACHE["nc"] = _build()
    return _CACHE["nc"]


def kernel(x, w_qkv, w_out, b_out, **kw):
    x = np.asarray(x, dtype=np.float32)
    w_qkv = np.asarray(w_qkv, dtype=np.float32)
    w_out = np.asarray(w_out, dtype=np.float32)
    b_out = np.asarray(b_out, dtype=np.float32)

    # fold q-scale into w_q (exact: power-of-two scale), then fp16-quantize
    w_qkv_s = w_qkv.copy()
    w_qkv_s[:, :HID] *= DH ** (-0.5)
    w_qkv_s = np.ascontiguousarray(w_qkv_s.astype(np.float16))
    # w_out [512, 256] -> [128, 4, 256] with [p, t, c] = w_out[t*128+p, c]
    w_out_r = np.ascontiguousarray(
        w_out.reshape(4, 128, C).transpose(1, 0, 2).astype(np.float16)
    )

    x4 = x.reshape(BATCH, D, C).astype(np.float16)
    in_maps = []
    for core in range(N_CORES):
        xs = np.ascontiguousarray(
            x4[core * BPC:(core + 1) * BPC].transpose(0, 2, 1)
        )  # [BPC, C, D] fp16
        in_maps.append({"xT": xs, "w_qkv": w_qkv_s, "w_out_r": w_out_r})

    nc = _get_nc()
    res = run_bass_kernel_spmd(nc, in_maps, core_ids=list(range(N_CORES)), **kw)
    y = np.concatenate([r["y"] for r in res.results], axis=0)  # [16, 4096, 256]
    y += b_out  # bias on host (broadcast over last axis)
    return y.reshape(BATCH, 64, 64, C)


# revision 29
# speedup vs baseline: 1.0342x; 1.0002x over previous
"""Channel-attention (per-head [64,64] score matrix) Trainium2 Bass kernel.

Math (per batch b of 16):
    qkv = x @ w_qkv                 # x [4096, 256], w_qkv [256, 1536]
    q,k,v = split(qkv); per head h (8 heads x 64 dim):
    sim_h = (q_h * 8^-1)^T @ k_h    # [64, 64]   (contracts spatial d=4096)
    attn_h = softmax(sim_h, axis=-1)
    out_h = v_h @ attn_h^T          # [4096, 64]
    y = concat(out_h) @ w_out + b_out

Distribution: data-parallel over batch — 8 cores x 2 batches each; weights
replicated; no collectives. The host pre-transposes x to [C, d] per batch so
every device matmul streams with large free dims, pre-folds the 1/8 q-scale
into w_q, pre-converts inputs to fp16 (all matmuls run fp16 x fp16 with fp32
PSUM accumulation; end-to-end rel-l2 ~1.6e-3 vs fp64 oracle), and adds the
output bias on the host (so y can DMA straight out of PSUM).

Device dataflow per batch (phases ordered so V-phase matmuls hide the
softmax latency on PE):
  QK:   q,k [d-chunk 128, 512each] (lhsT = xT d-chunk, rhs = w_qk cols, N=512)
  B:    sim[p] [128,128] per head-pair accumulates over 32 d-chunks
  V:    vT[m,d] = w_v.T @ xT       (lhsT = w_v chunk, rhs = xT d-cols, N=512)
  soft: rowmax (negated) -> exp(sim - max) with accum_out row-sums ->
        recip -> scale e rows by 1/s (so C1's PSUM drain is a plain copy)
  T:    PE-transpose e_p -> eT_p (C1's stationary operand)
  C1:   outT[i,d] = eT_h @ vT_h, two heads per PE pass (row/col split)
  C2:   y[d,c] = outT.T @ w_out, DMA'd to HBM directly from PSUM (fp32)
"""

import numpy as np

import concourse.bass as bass
import concourse.mybir as mybir
from concourse.bass_utils import run_bass_kernel_spmd
from concourse.masks import make_identity
from concourse.tile import TileContext


def _split_multi_waits(nc, limit=1):
    """Post-pass: the walrus build in this container rejects instructions
    carrying more than `limit` sync-waits ("Too many sync wait commands" in
    setupSyncWait). Tile attaches up to 3. Hoist the extras onto same-engine
    NoOp instructions inserted immediately before the owner — the engine
    sequencer executes them in order, so the ordering semantics are
    identical (single-wait instructions are what the rest of the Tile
    output uses, and those compile)."""
    n_split = 0
    for f in nc.m.functions:
        for blk in f.blocks:
            il = blk.instructions
            i = 0
            while i < len(il):
                inst = il[i]
                si = inst.sync_info
                waits = list(si.on_wait) if si is not None else []
                if len(waits) > limit:
                    si.on_wait = waits[:limit]
                    for w in waits[limit:]:
                        nop = mybir.InstNoOp(
                            name=f"I-waitsplit-{n_split}", ins=[], outs=[]
                        )
                        n_split += 1
                        nop.engine = inst.engine
                        nop.sync_info = mybir.SyncInfo(on_wait=[w], on_update=[])
                        il.insert(i, nop)
                        i += 1
                i += 1
    return nc


N_CORES = 8
BATCH = 16
BPC = BATCH // N_CORES  # batches per core
D = 4096  # spatial (64*64)
C = 256   # channels
HID = 512
HEADS = 8
DH = 64

F32 = mybir.dt.float32
F16 = mybir.dt.float16

_CACHE = {}


def _build():
    nc = bass.Bass()
    xT_d = nc.declare_dram_parameter("xT", [BPC, C, D], F16, isOutput=False)
    wqkv_d = nc.declare_dram_parameter("w_qkv", [C, 3 * HID], F16, isOutput=False)
    wout_d = nc.declare_dram_parameter("w_out_r", [128, 4, C], F16, isOutput=False)
    y_d = nc.declare_dram_parameter("y", [BPC, D, C], F32, isOutput=True)

    with TileContext(nc) as tc:
        with (
            tc.tile_pool(name="consts", bufs=1) as consts,
            tc.tile_pool(name="xt", bufs=2) as xt_pool,
            tc.tile_pool(name="vt", bufs=8) as vt_pool,
            tc.tile_pool(name="qk", bufs=6) as qk_pool,
            tc.tile_pool(name="eP", bufs=8) as e_pool,
            tc.tile_pool(name="stat", bufs=6) as stat_pool,
            tc.tile_pool(name="ot", bufs=8) as ot_pool,
            tc.tile_pool(name="ysb", bufs=4) as y_pool,
            tc.tile_pool(name="mm", bufs=6, space="PSUM") as mm_pool,
            tc.tile_pool(name="simp", bufs=2, space="PSUM") as sim_pool,
        ):
            # ---- constants ----
            # w_qkv split loads ordered by first use: w_q, then w_k, then
            # w_v / w_out (V and C2 run much later).
            w_sb = []
            for ci in range(2):
                w_t = consts.tile([128, 3 * HID], F16, name=f"w{ci}")
                w_sb.append(w_t)
            for ci in range(2):
                nc.sync.dma_start(
                    out=w_sb[ci][:, 0:HID],
                    in_=wqkv_d[ci * 128:(ci + 1) * 128, 0:HID],
                )
            wo_sb = consts.tile([128, 4, C], F16, name="wo")
            ident = consts.tile([128, 128], F32, name="ident")
            make_identity(nc, ident)

            for b in range(BPC):
                # ---- load xT (chunked so the first QK matmuls start early) --
                xt = []
                for ci in range(2):
                    x_t = xt_pool.tile([128, D], F16, name=f"xt{ci}", tag="xt")
                    xt.append(x_t)
                # first 512 cols arrive alone so QK d1=0..3 can start
                # early; w_k loads are interleaved after them (the k matmuls
                # trail the q matmuls by the pipeline skew anyway)
                chunks = [(0, 512)] + [(lo, lo + 896) for lo in range(512, D, 896)]
                for ki, (lo, hi) in enumerate(chunks):
                    hi = min(hi, D)
                    for ci in range(2):
                        nc.sync.dma_start(
                            out=xt[ci][:, lo:hi],
                            in_=xT_d[b, ci * 128:(ci + 1) * 128, lo:hi],
                        )
                    if b == 0 and ki == 0:
                        for ci in range(2):
                            nc.sync.dma_start(
                                out=w_sb[ci][:, HID:2 * HID],
                                in_=wqkv_d[ci * 128:(ci + 1) * 128, HID:2 * HID],
                            )

                # ---- phase QK + B ----
                # sim[p]: one PSUM bank per accumulation group (start=True
                # zeroes a whole 2KB zero-region per written partition, so
                # groups must not share a bank). Tile p = head pair
                # (2p, 2p+1): rows i (head 2p at 0:64, 2p+1 at 64:128),
                # cols j likewise; diag 64x64 blocks are the per-head sims.
                # sim_all [128, 256]: ONE psum bank holds all 8 per-head
                # accumulators — pair p at cols p*64:+64, head 2p at rows
                # 0:64, head 2p+1 at rows 64:128. The bank is zeroed by an
                # explicit memset and every matmul uses start=False
                # (accumulate) — order-independent, so the scheduler may
                # interleave the groups freely.
                sim_all = sim_pool.tile([128, 256], F32, name="sim_all", tag="simp")
                nc.vector.memset(sim_all, 0.0)
                def emit_b(qk_tile, d1):
                    # sim matmuls for the qk tile of iteration d1 (emitted one
                    # iteration late so the PSUM->SBUF copy latency hides
                    # under the next iteration's qk matmuls)
                    for p in range(4):
                        for par in range(2):
                            q_lo = p * 128 + par * 64
                            nc.tensor.matmul(
                                sim_all[par * 64:(par + 1) * 64, p * 64:(p + 1) * 64],
                                lhsT=qk_tile[:, q_lo:q_lo + 64],
                                rhs=qk_tile[:, 512 + q_lo:512 + q_lo + 64],
                                start=False,
                                stop=(d1 == 31),
                                skip_group_check=True,
                            )

                prev = None
                for d1 in range(32):
                    qps = mm_pool.tile([128, 512], F32, name="qps", tag="mm")
                    kps = mm_pool.tile([128, 512], F32, name="kps", tag="mm")
                    for ci in range(2):
                        nc.tensor.matmul(
                            qps,
                            lhsT=xt[ci][:, d1 * 128:(d1 + 1) * 128],
                            rhs=w_sb[ci][:, 0:HID],
                            start=(ci == 0),
                            stop=(ci == 1),
                        )
                    for ci in range(2):
                        nc.tensor.matmul(
                            kps,
                            lhsT=xt[ci][:, d1 * 128:(d1 + 1) * 128],
                            rhs=w_sb[ci][:, HID:2 * HID],
                            start=(ci == 0),
                            stop=(ci == 1),
                        )
                    qk = qk_pool.tile([128, 1024], F16, name="qk", tag="qk")
                    nc.any.tensor_copy(qk[:, 0:512], qps)
                    nc.any.tensor_copy(qk[:, 512:1024], kps)
                    if prev is not None:
                        emit_b(*prev)
                    prev = (qk, d1)

                # ---- phase V (PE work that hides softmax latency) ----
                # d5-outer so vt[0..3] become ready column-range by
                # column-range — C1's d5 loop can start at d5=0 early. The
                # first d5 iteration is emitted BEFORE the last deferred B
                # matmuls so the scheduler has PE work to cover the final
                # qk copy's latency.
                if b == 0:
                    # deferred weight loads (not needed until now)
                    for ci in range(2):
                        nc.sync.dma_start(
                            out=w_sb[ci][:, 2 * HID:3 * HID],
                            in_=wqkv_d[ci * 128:(ci + 1) * 128, 2 * HID:3 * HID],
                        )
                    nc.sync.dma_start(out=wo_sb, in_=wout_d[:, :, :])
                vt = []
                for m in range(4):
                    v_t = vt_pool.tile([128, D], F16, name=f"vt{m}", tag="vt")
                    vt.append(v_t)

                def emit_v(d5):
                    for m in range(4):
                        wv_lo = 2 * HID + m * 128
                        vps = mm_pool.tile([128, 512], F32, name="vps", tag="mm")
                        for ci in range(2):
                            nc.tensor.matmul(
                                vps,
                                lhsT=w_sb[ci][:, wv_lo:wv_lo + 128],
                                rhs=xt[ci][:, d5 * 512:(d5 + 1) * 512],
                                start=(ci == 0),
                                stop=(ci == 1),
                            )
                        nc.any.tensor_copy(vt[m][:, d5 * 512:(d5 + 1) * 512], vps)

                emit_b(*prev)
                for d5 in range(8):
                    emit_v(d5)

                # ---- softmax (DVE/ACT; overlaps V on PE) ----
                # head h: pair p=h//2, par=h%2; diag block of sim[p] at
                # rows/cols par*64:+64.
                m_t = stat_pool.tile([128, 4], F32, name="m_t", tag="stat")
                s_t = stat_pool.tile([128, 4], F32, name="s_t", tag="stat")
                r_t = stat_pool.tile([128, 4], F32, name="r_t", tag="stat")
                e_tiles = []
                for p in range(4):
                    e_p = e_pool.tile([128, 128], F32, name=f"e{p}", tag="e")
                    nc.gpsimd.memset(e_p, 0.0)
                    e_tiles.append(e_p)
                for h in range(HEADS):
                    par, p = h % 2, h // 2
                    rows = slice(par * 64, par * 64 + 64)
                    nc.vector.reduce_max(
                        out=m_t[rows, p:p + 1],
                        in_=sim_all[rows, p * 64:(p + 1) * 64],
                        axis=mybir.AxisListType.X,
                        negate=True,
                    )
                for h in range(HEADS):
                    par, p = h % 2, h // 2
                    rows = slice(par * 64, par * 64 + 64)
                    nc.scalar.activation(
                        out=e_tiles[p][rows, par * 64:par * 64 + 64],
                        in_=sim_all[rows, p * 64:(p + 1) * 64],
                        func=mybir.ActivationFunctionType.Exp,
                        bias=m_t[rows, p:p + 1],
                        scale=1.0,
                        accum_out=s_t[rows, p:p + 1],
                    )
                nc.vector.reciprocal(r_t, s_t)
                # attn = e / s: fold 1/s into e rows now (tiny [128,128]
                # tiles) instead of scaling every [128,512] C1 output.
                for p in range(4):
                    nc.vector.tensor_scalar_mul(
                        e_tiles[p], e_tiles[p], r_t[:, p:p + 1]
                    )

                # ---- transpose e -> eT (PE) ----
                eT_tiles = []
                for p in range(4):
                    etps = mm_pool.tile([128, 128], F32, name="etps", tag="mm")
                    nc.tensor.transpose(etps, e_tiles[p], ident)
                    eT_s = e_pool.tile([128, 128], F16, name=f"eT{p}", tag="eT")
                    nc.any.tensor_copy(eT_s, etps)
                    eT_tiles.append(eT_s)

                # ---- phase C: attention-apply + output projection ----
                def emit_c2(ot_tiles, d5):
                    # C2 for d5's ot tiles (emitted one d5 late so the ot
                    # copy latency hides under the next d5's C1 matmuls)
                    for d1 in range(4):
                        yps = mm_pool.tile([128, C], F32, name="yps", tag="mm")
                        for p4 in range(4):
                            nc.tensor.matmul(
                                yps,
                                lhsT=ot_tiles[p4][:, d1 * 128:(d1 + 1) * 128],
                                rhs=wo_sb[:, p4, :],
                                start=(p4 == 0),
                                stop=(p4 == 3),
                            )
                        ysb = y_pool.tile([128, C], F32, name="ysb", tag="ysb")
                        nc.any.tensor_copy(ysb, yps)
                        d_lo = d5 * 512 + d1 * 128
                        nc.sync.dma_start(out=y_d[b, d_lo:d_lo + 128, :], in_=ysb)

                prev_c = None
                for d5 in range(8):
                    ot_tiles = []
                    for p in range(4):
                        c1ps = mm_pool.tile([128, 512], F32, name="c1ps", tag="mm")
                        # eT_p is exactly block-diagonal (off-diag blocks are
                        # memset zeros), so one full-array K=128 matmul
                        # computes both heads: rows 0:64 of eT only meet
                        # vt rows 0:64 (head 2p), rows 64:128 only head 2p+1.
                        nc.tensor.matmul(
                            c1ps,
                            lhsT=eT_tiles[p],
                            rhs=vt[p][:, d5 * 512:(d5 + 1) * 512],
                            start=True,
                            stop=True,
                        )
                        ot = ot_pool.tile([128, 512], F16, name=f"ot{p}", tag="ot")
                        nc.any.tensor_copy(ot, c1ps)
                        ot_tiles.append(ot)
                    if prev_c is not None:
                        emit_c2(*prev_c)
                    prev_c = (ot_tiles, d5)
                emit_c2(*prev_c)
    return _split_multi_waits(nc)


def _get_nc():
    if "nc" not in _CACHE:
        _CACHE["nc"] = _build()
    return _CACHE["nc"]


def kernel(x, w_qkv, w_out, b_out, **kw):
    x = np.asarray(x, dtype=np.float32)
    w_qkv = np.asarray(w_qkv, dtype=np.float32)
    w_out = np.asarray(w_out, dtype=np.float32)
    b_out = np.asarray(b_out, dtype=np.float32)

    # fold q-scale into w_q (exact: power-of-two scale), then fp16-quantize
    w_qkv_s = w_qkv.copy()
    w_qkv_s[:, :HID] *= DH ** (-0.5)
    w_qkv_s = np.ascontiguousarray(w_qkv_s.astype(np.float16))
    # w_out [512, 256] -> [128, 4, 256] with [p, t, c] = w_out[t*128+p, c]
    w_out_r = np.ascontiguousarray(
        w_out.reshape(4, 128, C).transpose(1, 0, 2).astype(np.float16)
    )

    x4 = x.reshape(BATCH, D, C).astype(np.float16)
    in_maps = []
    for core in range(N_CORES):
        xs = np.ascontiguousarray(
            x4[core * BPC:(core + 1) * BPC].transpose(0, 2, 1)
        )  # [BPC, C, D] fp16
        in_maps.append({"xT": xs, "w_qkv": w_qkv_s, "w_out_r": w_out_r})

    nc = _get_nc()
    res = run_bass_kernel_spmd(nc, in_maps, core_ids=list(range(N_CORES)), **kw)
    y = np.concatenate([r["y"] for r in res.results], axis=0)  # [16, 4096, 256]
    y += b_out  # bias on host (broadcast over last axis)
    return y.reshape(BATCH, 64, 64, C)


# revision 30
# speedup vs baseline: 1.0349x; 1.0006x over previous
"""Channel-attention (per-head [64,64] score matrix) Trainium2 Bass kernel.

Math (per batch b of 16):
    qkv = x @ w_qkv                 # x [4096, 256], w_qkv [256, 1536]
    q,k,v = split(qkv); per head h (8 heads x 64 dim):
    sim_h = (q_h * 8^-1)^T @ k_h    # [64, 64]   (contracts spatial d=4096)
    attn_h = softmax(sim_h, axis=-1)
    out_h = v_h @ attn_h^T          # [4096, 64]
    y = concat(out_h) @ w_out + b_out

Distribution: data-parallel over batch — 8 cores x 2 batches each; weights
replicated; no collectives. The host pre-transposes x to [C, d] per batch so
every device matmul streams with large free dims, pre-folds the 1/8 q-scale
into w_q, pre-converts inputs to fp16 (all matmuls run fp16 x fp16 with fp32
PSUM accumulation; end-to-end rel-l2 ~1.6e-3 vs fp64 oracle), and adds the
output bias on the host (so y can DMA straight out of PSUM).

Device dataflow per batch (phases ordered so V-phase matmuls hide the
softmax latency on PE):
  QK:   q,k [d-chunk 128, 512each] (lhsT = xT d-chunk, rhs = w_qk cols, N=512)
  B:    sim[p] [128,128] per head-pair accumulates over 32 d-chunks
  V:    vT[m,d] = w_v.T @ xT       (lhsT = w_v chunk, rhs = xT d-cols, N=512)
  soft: rowmax (negated) -> exp(sim - max) with accum_out row-sums ->
        recip -> scale e rows by 1/s (so C1's PSUM drain is a plain copy)
  T:    PE-transpose e_p -> eT_p (C1's stationary operand)
  C1:   outT[i,d] = eT_h @ vT_h, two heads per PE pass (row/col split)
  C2:   y[d,c] = outT.T @ w_out, DMA'd to HBM directly from PSUM (fp32)
"""

import numpy as np

import concourse.bass as bass
import concourse.mybir as mybir
from concourse.bass_utils import run_bass_kernel_spmd
from concourse.masks import make_identity
from concourse.tile import TileContext


def _split_multi_waits(nc, limit=1):
    """Post-pass: the walrus build in this container rejects instructions
    carrying more than `limit` sync-waits ("Too many sync wait commands" in
    setupSyncWait). Tile attaches up to 3. Hoist the extras onto same-engine
    NoOp instructions inserted immediately before the owner — the engine
    sequencer executes them in order, so the ordering semantics are
    identical (single-wait instructions are what the rest of the Tile
    output uses, and those compile)."""
    drain_engines = [
        mybir.EngineType.PE,
        mybir.EngineType.DVE,
        mybir.EngineType.Activation,
        mybir.EngineType.Pool,
        mybir.EngineType.SP,
    ]
    n_split = 0
    for f in nc.m.functions:
        for blk in f.blocks:
            il = blk.instructions
            i = 0
            while i < len(il):
                inst = il[i]
                si = inst.sync_info
                waits = list(si.on_wait) if si is not None else []
                if len(waits) > limit:
                    si.on_wait = waits[:limit]
                    # The kernel-tail drain aggregates one wait per logical
                    # processor; those can wait in parallel across engines
                    # (the all-engine barrier that follows orders them before
                    # the semaphore clears). Mid-program instructions keep
                    # their extras on their own engine to preserve ordering.
                    is_drain = type(inst).__name__ == "InstDrain"
                    for k, w in enumerate(waits[limit:]):
                        nop = mybir.InstNoOp(
                            name=f"I-waitsplit-{n_split}", ins=[], outs=[]
                        )
                        n_split += 1
                        nop.engine = (
                            drain_engines[k % len(drain_engines)]
                            if is_drain else inst.engine
                        )
                        nop.sync_info = mybir.SyncInfo(on_wait=[w], on_update=[])
                        il.insert(i, nop)
                        i += 1
                i += 1
    return nc


N_CORES = 8
BATCH = 16
BPC = BATCH // N_CORES  # batches per core
D = 4096  # spatial (64*64)
C = 256   # channels
HID = 512
HEADS = 8
DH = 64

F32 = mybir.dt.float32
F16 = mybir.dt.float16

_CACHE = {}


def _build():
    nc = bass.Bass()
    xT_d = nc.declare_dram_parameter("xT", [BPC, C, D], F16, isOutput=False)
    wqkv_d = nc.declare_dram_parameter("w_qkv", [C, 3 * HID], F16, isOutput=False)
    wout_d = nc.declare_dram_parameter("w_out_r", [128, 4, C], F16, isOutput=False)
    y_d = nc.declare_dram_parameter("y", [BPC, D, C], F32, isOutput=True)

    with TileContext(nc) as tc:
        with (
            tc.tile_pool(name="consts", bufs=1) as consts,
            tc.tile_pool(name="xt", bufs=2) as xt_pool,
            tc.tile_pool(name="vt", bufs=8) as vt_pool,
            tc.tile_pool(name="qk", bufs=6) as qk_pool,
            tc.tile_pool(name="eP", bufs=8) as e_pool,
            tc.tile_pool(name="stat", bufs=6) as stat_pool,
            tc.tile_pool(name="ot", bufs=8) as ot_pool,
            tc.tile_pool(name="ysb", bufs=4) as y_pool,
            tc.tile_pool(name="mm", bufs=6, space="PSUM") as mm_pool,
            tc.tile_pool(name="simp", bufs=2, space="PSUM") as sim_pool,
        ):
            # ---- constants ----
            # w_qkv split loads ordered by first use: w_q, then w_k, then
            # w_v / w_out (V and C2 run much later).
            w_sb = []
            for ci in range(2):
                w_t = consts.tile([128, 3 * HID], F16, name=f"w{ci}")
                w_sb.append(w_t)
            for ci in range(2):
                nc.sync.dma_start(
                    out=w_sb[ci][:, 0:HID],
                    in_=wqkv_d[ci * 128:(ci + 1) * 128, 0:HID],
                )
            wo_sb = consts.tile([128, 4, C], F16, name="wo")
            ident = consts.tile([128, 128], F32, name="ident")
            make_identity(nc, ident)

            for b in range(BPC):
                # ---- load xT (chunked so the first QK matmuls start early) --
                xt = []
                for ci in range(2):
                    x_t = xt_pool.tile([128, D], F16, name=f"xt{ci}", tag="xt")
                    xt.append(x_t)
                # first 512 cols arrive alone so QK d1=0..3 can start
                # early; w_k loads are interleaved after them (the k matmuls
                # trail the q matmuls by the pipeline skew anyway)
                chunks = [(0, 512)] + [(lo, lo + 896) for lo in range(512, D, 896)]
                for ki, (lo, hi) in enumerate(chunks):
                    hi = min(hi, D)
                    for ci in range(2):
                        nc.sync.dma_start(
                            out=xt[ci][:, lo:hi],
                            in_=xT_d[b, ci * 128:(ci + 1) * 128, lo:hi],
                        )
                    if b == 0 and ki == 0:
                        for ci in range(2):
                            nc.sync.dma_start(
                                out=w_sb[ci][:, HID:2 * HID],
                                in_=wqkv_d[ci * 128:(ci + 1) * 128, HID:2 * HID],
                            )

                # ---- phase QK + B ----
                # sim[p]: one PSUM bank per accumulation group (start=True
                # zeroes a whole 2KB zero-region per written partition, so
                # groups must not share a bank). Tile p = head pair
                # (2p, 2p+1): rows i (head 2p at 0:64, 2p+1 at 64:128),
                # cols j likewise; diag 64x64 blocks are the per-head sims.
                # sim_all [128, 256]: ONE psum bank holds all 8 per-head
                # accumulators — pair p at cols p*64:+64, head 2p at rows
                # 0:64, head 2p+1 at rows 64:128. The bank is zeroed by an
                # explicit memset and every matmul uses start=False
                # (accumulate) — order-independent, so the scheduler may
                # interleave the groups freely.
                sim_all = sim_pool.tile([128, 256], F32, name="sim_all", tag="simp")
                nc.vector.memset(sim_all, 0.0)
                def emit_b(qk_tile, d1):
                    # sim matmuls for the qk tile of iteration d1 (emitted one
                    # iteration late so the PSUM->SBUF copy latency hides
                    # under the next iteration's qk matmuls)
                    for p in range(4):
                        for par in range(2):
                            q_lo = p * 128 + par * 64
                            nc.tensor.matmul(
                                sim_all[par * 64:(par + 1) * 64, p * 64:(p + 1) * 64],
                                lhsT=qk_tile[:, q_lo:q_lo + 64],
                                rhs=qk_tile[:, 512 + q_lo:512 + q_lo + 64],
                                start=False,
                                stop=(d1 == 31),
                                skip_group_check=True,
                            )

                prev = None
                for d1 in range(32):
                    qps = mm_pool.tile([128, 512], F32, name="qps", tag="mm")
                    kps = mm_pool.tile([128, 512], F32, name="kps", tag="mm")
                    for ci in range(2):
                        nc.tensor.matmul(
                            qps,
                            lhsT=xt[ci][:, d1 * 128:(d1 + 1) * 128],
                            rhs=w_sb[ci][:, 0:HID],
                            start=(ci == 0),
                            stop=(ci == 1),
                        )
                    for ci in range(2):
                        nc.tensor.matmul(
                            kps,
                            lhsT=xt[ci][:, d1 * 128:(d1 + 1) * 128],
                            rhs=w_sb[ci][:, HID:2 * HID],
                            start=(ci == 0),
                            stop=(ci == 1),
                        )
                    qk = qk_pool.tile([128, 1024], F16, name="qk", tag="qk")
                    nc.any.tensor_copy(qk[:, 0:512], qps)
                    nc.any.tensor_copy(qk[:, 512:1024], kps)
                    if prev is not None:
                        emit_b(*prev)
                    prev = (qk, d1)

                # ---- phase V (PE work that hides softmax latency) ----
                # d5-outer so vt[0..3] become ready column-range by
                # column-range — C1's d5 loop can start at d5=0 early. The
                # first d5 iteration is emitted BEFORE the last deferred B
                # matmuls so the scheduler has PE work to cover the final
                # qk copy's latency.
                if b == 0:
                    # deferred weight loads (not needed until now)
                    for ci in range(2):
                        nc.sync.dma_start(
                            out=w_sb[ci][:, 2 * HID:3 * HID],
                            in_=wqkv_d[ci * 128:(ci + 1) * 128, 2 * HID:3 * HID],
                        )
                    nc.sync.dma_start(out=wo_sb, in_=wout_d[:, :, :])
                vt = []
                for m in range(4):
                    v_t = vt_pool.tile([128, D], F16, name=f"vt{m}", tag="vt")
                    vt.append(v_t)

                def emit_v(d5):
                    for m in range(4):
                        wv_lo = 2 * HID + m * 128
                        vps = mm_pool.tile([128, 512], F32, name="vps", tag="mm")
                        for ci in range(2):
                            nc.tensor.matmul(
                                vps,
                                lhsT=w_sb[ci][:, wv_lo:wv_lo + 128],
                                rhs=xt[ci][:, d5 * 512:(d5 + 1) * 512],
                                start=(ci == 0),
                                stop=(ci == 1),
                            )
                        nc.any.tensor_copy(vt[m][:, d5 * 512:(d5 + 1) * 512], vps)

                emit_b(*prev)
                for d5 in range(8):
                    emit_v(d5)

                # ---- softmax (DVE/ACT; overlaps V on PE) ----
                # head h: pair p=h//2, par=h%2; diag block of sim[p] at
                # rows/cols par*64:+64.
                m_t = stat_pool.tile([128, 4], F32, name="m_t", tag="stat")
                s_t = stat_pool.tile([128, 4], F32, name="s_t", tag="stat")
                r_t = stat_pool.tile([128, 4], F32, name="r_t", tag="stat")
                e_tiles = []
                for p in range(4):
                    e_p = e_pool.tile([128, 128], F32, name=f"e{p}", tag="e")
                    nc.gpsimd.memset(e_p, 0.0)
                    e_tiles.append(e_p)
                for h in range(HEADS):
                    par, p = h % 2, h // 2
                    rows = slice(par * 64, par * 64 + 64)
                    nc.vector.reduce_max(
                        out=m_t[rows, p:p + 1],
                        in_=sim_all[rows, p * 64:(p + 1) * 64],
                        axis=mybir.AxisListType.X,
                        negate=True,
                    )
                for h in range(HEADS):
                    par, p = h % 2, h // 2
                    rows = slice(par * 64, par * 64 + 64)
                    nc.scalar.activation(
                        out=e_tiles[p][rows, par * 64:par * 64 + 64],
                        in_=sim_all[rows, p * 64:(p + 1) * 64],
                        func=mybir.ActivationFunctionType.Exp,
                        bias=m_t[rows, p:p + 1],
                        scale=1.0,
                        accum_out=s_t[rows, p:p + 1],
                    )
                nc.vector.reciprocal(r_t, s_t)
                # attn = e / s: fold 1/s into e rows now (tiny [128,128]
                # tiles) instead of scaling every [128,512] C1 output.
                for p in range(4):
                    nc.vector.tensor_scalar_mul(
                        e_tiles[p], e_tiles[p], r_t[:, p:p + 1]
                    )

                # ---- transpose e -> eT (PE) ----
                eT_tiles = []
                for p in range(4):
                    etps = mm_pool.tile([128, 128], F32, name="etps", tag="mm")
                    nc.tensor.transpose(etps, e_tiles[p], ident)
                    eT_s = e_pool.tile([128, 128], F16, name=f"eT{p}", tag="eT")
                    nc.any.tensor_copy(eT_s, etps)
                    eT_tiles.append(eT_s)

                # ---- phase C: attention-apply + output projection ----
                def emit_c2(ot_tiles, d5):
                    # C2 for d5's ot tiles (emitted one d5 late so the ot
                    # copy latency hides under the next d5's C1 matmuls)
                    for d1 in range(4):
                        yps = mm_pool.tile([128, C], F32, name="yps", tag="mm")
                        for p4 in range(4):
                            nc.tensor.matmul(
                                yps,
                                lhsT=ot_tiles[p4][:, d1 * 128:(d1 + 1) * 128],
                                rhs=wo_sb[:, p4, :],
                                start=(p4 == 0),
                                stop=(p4 == 3),
                            )
                        ysb = y_pool.tile([128, C], F32, name="ysb", tag="ysb")
                        nc.any.tensor_copy(ysb, yps)
                        d_lo = d5 * 512 + d1 * 128
                        nc.sync.dma_start(out=y_d[b, d_lo:d_lo + 128, :], in_=ysb)

                prev_c = None
                for d5 in range(8):
                    ot_tiles = []
                    for p in range(4):
                        c1ps = mm_pool.tile([128, 512], F32, name="c1ps", tag="mm")
                        # eT_p is exactly block-diagonal (off-diag blocks are
                        # memset zeros), so one full-array K=128 matmul
                        # computes both heads: rows 0:64 of eT only meet
                        # vt rows 0:64 (head 2p), rows 64:128 only head 2p+1.
                        nc.tensor.matmul(
                            c1ps,
                            lhsT=eT_tiles[p],
                            rhs=vt[p][:, d5 * 512:(d5 + 1) * 512],
                            start=True,
                            stop=True,
                        )
                        ot = ot_pool.tile([128, 512], F16, name=f"ot{p}", tag="ot")
                        nc.any.tensor_copy(ot, c1ps)
                        ot_tiles.append(ot)
                    if prev_c is not None:
                        emit_c2(*prev_c)
                    prev_c = (ot_tiles, d5)
                emit_c2(*prev_c)
    return _split_multi_waits(nc)


def _get_nc():
    if "nc" not in _CACHE:
        _CACHE["nc"] = _build()
    return _CACHE["nc"]


def kernel(x, w_qkv, w_out, b_out, **kw):
    x = np.asarray(x, dtype=np.float32)
    w_qkv = np.asarray(w_qkv, dtype=np.float32)
    w_out = np.asarray(w_out, dtype=np.float32)
    b_out = np.asarray(b_out, dtype=np.float32)

    # fold q-scale into w_q (exact: power-of-two scale), then fp16-quantize
    w_qkv_s = w_qkv.copy()
    w_qkv_s[:, :HID] *= DH ** (-0.5)
    w_qkv_s = np.ascontiguousarray(w_qkv_s.astype(np.float16))
    # w_out [512, 256] -> [128, 4, 256] with [p, t, c] = w_out[t*128+p, c]
    w_out_r = np.ascontiguousarray(
        w_out.reshape(4, 128, C).transpose(1, 0, 2).astype(np.float16)
    )

    x4 = x.reshape(BATCH, D, C).astype(np.float16)
    in_maps = []
    for core in range(N_CORES):
        xs = np.ascontiguousarray(
            x4[core * BPC:(core + 1) * BPC].transpose(0, 2, 1)
        )  # [BPC, C, D] fp16
        in_maps.append({"xT": xs, "w_qkv": w_qkv_s, "w_out_r": w_out_r})

    nc = _get_nc()
    res = run_bass_kernel_spmd(nc, in_maps, core_ids=list(range(N_CORES)), **kw)
    y = np.concatenate([r["y"] for r in res.results], axis=0)  # [16, 4096, 256]
    y += b_out  # bias on host (broadcast over last axis)
    return y.reshape(BATCH, 64, 64, C)
